# revision 50
# baseline (speedup 1.0000x reference)
"""Trainium2 Bass kernel for nn_Attention_9560597201123.

Full multi-head attention (B=4, N=2048, E=1024, H=16, D=64), f32 reference.

Sharding (tensor parallel over heads + data parallel over batch):
  8 cores = (batch b in 0..4) x (head half hh in 0..2). Each core:
    - receives x[b].T, the qkv weight columns for its 8 heads, and the
      proj weight rows for its 8 heads
    - computes q/k/v projections for its 8 heads over the FULL sequence,
      attention for those heads, and a PARTIAL output projection
      (contraction over its 512 e-dims), returned in bf16 [2048, 1024]
  Host sums the two partials per batch and adds proj_b.

Device layout notes:
  - scores are computed transposed (S^T: keys on partitions, queries free)
    so that P^T = exp(S^T) feeds the PV matmul directly (contraction = keys)
  - the two heads of a pair sit at partitions 0:64 / 64:128 so the two
    score matmuls run concurrently in the PE array (row tiling)
  - softmax normalizer: v is extended with a ones column (lhsT M=65), so
    the PV matmul's partition 64 accumulates the row sums for free
  - k bias is dropped entirely: it shifts each score row by a constant,
    which softmax cancels
  - optionally, exp for some key-chunks is computed on the Vector engine
    via a Schraudolph-style bit trick (affine -> int16 bits -> bf16),
    relieving the Scalar engine which otherwise paces the attention phase
  - all TensorEngine matmuls run in bf16 (1 cycle/row); accumulation f32
"""

import numpy as np
import ml_dtypes

P = 128
SEQ = 2048
E = 1024
HL = 8  # local heads per core
NPAIR = 4  # local head pairs
D = 64
KC = 16  # key chunks of 128
EC = 8  # e_in chunks of 128
NQB = 4  # query blocks of 512
SCALE = D ** -0.5  # 0.125

# kc chunks whose exp runs on the Vector engine (Schraudolph bit trick)
SCHRAUDOLPH_KC = ()
# exp(x*SCALE) ~ bf16(bits = round(x * SCH_A + SCH_B)) as int16
SCH_A = SCALE * 128.0 / float(np.log(2.0))
SCH_B = 127.0 * 128.0 - 5.5

_NC = None


def build_nc():
    global _NC
    if _NC is not None:
        return _NC

    import concourse.bass as bass  # noqa: F401
    import concourse.mybir as mybir
    import concourse.tile as tile
    from concourse import bacc

    BF = mybir.dt.bfloat16
    F32 = mybir.dt.float32
    I16 = mybir.dt.int16
    EXP = mybir.ActivationFunctionType.Exp
    LOG = mybir.ActivationFunctionType.Ln
    COPY = mybir.ActivationFunctionType.Identity
    ADD = mybir.AluOpType.add
    MULT = mybir.AluOpType.mult

    nc = bacc.Bacc("TRN2", target_bir_lowering=False, debug=False, num_devices=8)

    xt_d = nc.dram_tensor("xt", [E, SEQ], BF, kind="ExternalInput").ap()
    wqkv_d = nc.dram_tensor("wqkv", [E, 3 * 512], BF, kind="ExternalInput").ap()
    bq_d = nc.dram_tensor("bq", [512], F32, kind="ExternalInput").ap()
    bv_bf_d = nc.dram_tensor("bv_bf", [512], BF, kind="ExternalInput").ap()
    wp_d = nc.dram_tensor("wp", [512, E], BF, kind="ExternalInput").ap()
    out_d = nc.dram_tensor("out", [SEQ, E], BF, kind="ExternalOutput").ap()

    wqkv_r = wqkv_d.rearrange("(o p) c -> p o c", p=P)

    with tile.TileContext(nc) as tc:
        with (
            tc.tile_pool(name="persist", bufs=1) as persist,
            tc.tile_pool(name="wstream", bufs=2) as wstream,
            tc.tile_pool(name="ptpool", bufs=6) as ptpool,
            tc.tile_pool(name="asbp", bufs=5) as asbp,
            tc.tile_pool(name="small", bufs=6) as small,
            tc.tile_pool(name="proj_ps", bufs=2, space="PSUM") as proj_ps,
            tc.tile_pool(name="acc_ps", bufs=2, space="PSUM") as acc_ps,
            tc.tile_pool(name="sc_ps", bufs=2, space="PSUM") as sc_ps,
        ):
            # ---- persistent tiles + input DMA ----
            # order matters: the first matmuls need wq0 + xt chunk 0, so those
            # DMAs go first on the sync queue
            xt_r = xt_d.rearrange("(o p) s -> p o s", p=P)
            xt3 = persist.tile([P, EC, SEQ], BF, tag="xt")
            wq0 = wstream.tile([P, EC, P], BF, tag="wq")
            nc.sync.dma_start(wq0[:], wqkv_r[:, :, 0:P])
            nc.sync.dma_start(xt3[:, :, 0:512], xt_r[:, :, 0:512])
            wk0 = wstream.tile([P, EC, P], BF, tag="wk")
            nc.sync.dma_start(wk0[:], wqkv_r[:, :, 512 : 512 + P])
            # spread the remaining xt chunks over two queues (sync + scalar)
            nc.scalar.dma_start(xt3[:, :, 512:1024], xt_r[:, :, 512:1024])
            nc.sync.dma_start(xt3[:, :, 1024:1536], xt_r[:, :, 1024:1536])
            nc.scalar.dma_start(xt3[:, :, 1536:2048], xt_r[:, :, 1536:2048])
            xt = [xt3[:, ec, :] for ec in range(EC)]

            vx4 = []
            for sm in range(KC):
                t = persist.tile([P, HL * 65], BF, tag=f"vx{sm}", name=f"vx{sm}").rearrange(
                    "p (h c) -> p h c", c=65
                )
                nc.vector.memset(t[:, :, 64], 1.0)
                vx4.append(t)

            kt = [persist.tile([P, SEQ], BF, tag=f"kt{p}", name=f"kt{p}") for p in range(NPAIR)]
            qt = [persist.tile([P, SEQ], BF, tag=f"qt{p}", name=f"qt{p}") for p in range(NPAIR)]
            # one aT tile per (pair, query block): a single [P, SEQ] tile would
            # make every projection chunk wait on the LATEST normalize chain
            # writing anywhere in it (tile-granular dependencies)
            aT = [
                [
                    persist.tile([P, 512], BF, tag=f"aT{p}_{qb}", name=f"aT{p}_{qb}")
                    for qb in range(NQB)
                ]
                for p in range(NPAIR)
            ]

            bq_t = persist.tile([P, NPAIR], F32, tag="bq_t")
            nc.gpsimd.dma_start(bq_t[:], bq_d.rearrange("(o p) -> p o", p=P))

            bv_row = persist.tile([1, 512], BF, tag="bv_row")
            nc.gpsimd.dma_start(bv_row[:], bv_bf_d[None])
            bv_bc = persist.tile([P, 512], BF, tag="bv_bc")
            nc.gpsimd.partition_broadcast(bv_bc[:], bv_row[:])

            # wv goes on the scalar queue before pw (it is needed much sooner)
            wv0 = persist.tile([P, EC, 512], BF, tag="wv")
            nc.scalar.dma_start(wv0[:], wqkv_r[:, :, 1024:1536])
            pw = persist.tile([P, NPAIR, E], BF, tag="pw")
            nc.scalar.dma_start(pw[:], wp_d.rearrange("(o p) c -> p o c", p=P))

            # ---- projections ----
            def emit_kq(p, wq=None, wk=None):
                if wq is None:
                    wq = wstream.tile([P, EC, P], BF, tag="wq")
                    nc.sync.dma_start(wq[:], wqkv_r[:, :, p * P : (p + 1) * P])
                for s in range(4):
                    ssl = slice(s * 512, (s + 1) * 512)
                    ps = proj_ps.tile([P, 512], F32, tag="ps512")
                    for ec in range(EC):
                        nc.tensor.matmul(
                            ps[:],
                            lhsT=wq[:, ec, :],
                            rhs=xt[ec][:, ssl],
                            start=(ec == 0),
                            stop=(ec == EC - 1),
                        )
                    nc.scalar.activation(
                        out=qt[p][:, ssl], in_=ps[:], func=COPY, bias=bq_t[:, p : p + 1]
                    )
                if wk is None:
                    wk = wstream.tile([P, EC, P], BF, tag="wk")
                    nc.sync.dma_start(
                        wk[:], wqkv_r[:, :, 512 + p * P : 512 + (p + 1) * P]
                    )
                for s in range(4):
                    ssl = slice(s * 512, (s + 1) * 512)
                    ps = proj_ps.tile([P, 512], F32, tag="ps512")
                    for ec in range(EC):
                        nc.tensor.matmul(
                            ps[:],
                            lhsT=wk[:, ec, :],
                            rhs=xt[ec][:, ssl],
                            start=(ec == 0),
                            stop=(ec == EC - 1),
                        )
                    # no k bias: softmax cancels a per-row constant
                    nc.scalar.activation(
                        out=kt[p][:, ssl], in_=ps[:], func=COPY, bias=0.0
                    )

            def emit_v():
                wv = wv0
                for sm in range(KC):
                    ps = proj_ps.tile([P, 512], F32, tag="ps512")
                    for ec in range(EC):
                        nc.tensor.matmul(
                            ps[:],
                            lhsT=xt[ec][:, sm * P : (sm + 1) * P],
                            rhs=wv[:, ec, :],
                            start=(ec == 0),
                            stop=(ec == EC - 1),
                        )
                    nc.vector.tensor_tensor(
                        out=vx4[sm][:, :, 0:64],
                        in0=ps[:].rearrange("p (h c) -> p h c", c=64),
                        in1=bv_bc[:].rearrange("p (h c) -> p h c", c=64),
                        op=ADD,
                    )

            # ---- attention ----
            pending_norm = []

            def flush_norm():
                for fn in pending_norm:
                    fn()
                pending_norm.clear()

            def emit_attention(p, qb):
                # flush the previous block's normalize chain FIRST: its inputs
                # were evicted a block ago, so it runs on the DVE right away,
                # well before this block's evicts (which wait for this block's
                # last PV matmul) — chains complete ~10us after their block
                flush_norm()
                qsl = slice(qb * 512, (qb + 1) * 512)
                accA = acc_ps.tile([65, 512], F32, tag="acc")
                accB = acc_ps.tile([65, 512], F32, tag="acc")
                for kc in range(KC):
                    sc = sc_ps.tile([P, 1024], F32, tag="sc")
                    nc.tensor.matmul(
                        sc[:, 0:512],
                        lhsT=kt[p][0:64, kc * P : (kc + 1) * P],
                        rhs=qt[p][0:64, qsl],
                    )
                    nc.tensor.matmul(
                        sc[:, 512:1024],
                        lhsT=kt[p][64:P, kc * P : (kc + 1) * P],
                        rhs=qt[p][64:P, qsl],
                    )
                    pt = ptpool.tile([P, 1024], BF, tag="pt")
                    if kc in SCHRAUDOLPH_KC:
                        nc.vector.tensor_scalar(
                            out=pt[:].bitcast(I16),
                            in0=sc[:],
                            scalar1=SCH_A,
                            scalar2=SCH_B,
                            op0=MULT,
                            op1=ADD,
                        )
                    else:
                        nc.scalar.activation(out=pt[:], in_=sc[:], func=EXP, scale=SCALE)
                    nc.tensor.matmul(
                        accA[:],
                        lhsT=vx4[kc][:, 2 * p, :],
                        rhs=pt[:, 0:512],
                        start=(kc == 0),
                        stop=(kc == KC - 1),
                    )
                    nc.tensor.matmul(
                        accB[:],
                        lhsT=vx4[kc][:, 2 * p + 1, :],
                        rhs=pt[:, 512:1024],
                        start=(kc == 0),
                        stop=(kc == KC - 1),
                    )
                # evict accumulators (frees the PSUM slots for the next block)
                asbs = []
                for acc in (accA, accB):
                    asb = asbp.tile([65, 512], F32, tag="asb")
                    nc.vector.tensor_copy(out=asb[:], in_=acc[:])
                    asbs.append(asb)

                def norm(p=p, qb=qb, asbs=asbs):
                    rss = []
                    for asb in asbs:
                        rs = small.tile([1, 512], F32, tag="rs")
                        nc.vector.reciprocal(rs[:], asb[64:65, :])
                        rss.append(rs)
                    # gpsimd runs ONLY partition_broadcast ops (mixing op
                    # families on gpsimd swaps microcode libraries at ~6us a
                    # pop); the multiplies stay on the DVE
                    Rs = []
                    for rs in rss:
                        R = small.tile([64, 512], F32, tag="R")
                        nc.gpsimd.partition_broadcast(R[:], rs[:])
                        Rs.append(R)
                    nc.vector.tensor_tensor(
                        out=aT[p][qb][0:64, :], in0=asbs[0][0:64, :], in1=Rs[0][:], op=MULT
                    )
                    tmpb = small.tile([64, 512], BF, tag="tmpb")
                    nc.vector.tensor_tensor(
                        out=tmpb[:], in0=asbs[1][0:64, :], in1=Rs[1][:], op=MULT
                    )
                    # gpsimd queue, NOT sync: the sync queue carries the
                    # projection output DMAs, and a shared FIFO queue would
                    # chain proj -> out-DMA -> this DMA -> next proj serially
                    nc.gpsimd.dma_start(aT[p][qb][64:P, :], tmpb[:])

                pending_norm.append(norm)

            # ---- partial output projection, one 128-query chunk at a time ----
            # the PSUM eviction (CAST on the DVE) waits on the chunk's matmuls;
            # emitting it inline would block the DVE queue behind the PE and
            # phase-lock the normalize chains one block late, so it is
            # deferred to the start of the next attention block
            pending_cast = []

            def flush_cast():
                for fn in pending_cast:
                    fn()
                pending_cast.clear()

            def emit_proj_chunk(qc):
                qbq, col = divmod(qc, 4)
                for ncol in range(2):
                    nsl = slice(ncol * 512, (ncol + 1) * 512)
                    yps = proj_ps.tile([P, 512], F32, tag="ps512")
                    for p in range(NPAIR):
                        nc.tensor.matmul(
                            yps[:],
                            lhsT=aT[p][qbq][:, col * P : (col + 1) * P],
                            rhs=pw[:, p, nsl],
                            start=(p == 0),
                            stop=(p == NPAIR - 1),
                        )

                    def cast(qc=qc, nsl=nsl, yps=yps):
                        ysb = small.tile([P, 512], BF, tag="ysb")
                        nc.vector.tensor_copy(out=ysb[:], in_=yps[:])
                        nc.sync.dma_start(out_d[qc * P : (qc + 1) * P, nsl], ysb[:])

                    pending_cast.append(cast)

            # projection chunks trail the attention stream by 5 blocks: chunk
            # c needs the normalize chains of its whole query block (gating
            # block 4*(c//4)+3), so c is emitted after attention block c+5 —
            # a ~2-block margin over the chain latency
            emit_kq(0, wq=wq0, wk=wk0)
            emit_v()
            # proj chunks trail the attention stream by 6 blocks: the gating
            # normalize chain needs ~15us past its block's last PV, while the
            # PE (running ahead on buffered exps) reaches a chunk ~10us after
            # the preceding block — two blocks of lag gives ~35us of margin
            block = 0
            for qb in range(NQB):
                for p in range(NPAIR):
                    emit_attention(p, qb)
                    flush_cast()
                    if qb == 0 and p < NPAIR - 1:
                        emit_kq(p + 1)
                    c = block - 6
                    if 0 <= c < 4 * (NQB - 1):
                        emit_proj_chunk(c)
                    block += 1
            emit_proj_chunk(10)
            emit_proj_chunk(11)
            flush_norm()
            for c in range(4 * (NQB - 1), 4 * NQB):
                flush_cast()
                emit_proj_chunk(c)
            flush_cast()

    nc.finalize()
    _NC = nc
    return nc


def make_in_maps(x, qkv_w, qkv_b, proj_w, proj_b):
    bf16 = ml_dtypes.bfloat16
    x = np.asarray(x, dtype=np.float32)
    qkv_w = np.asarray(qkv_w, dtype=np.float32)
    qkv_b = np.asarray(qkv_b, dtype=np.float32)
    proj_w = np.asarray(proj_w, dtype=np.float32)
    xts = [np.ascontiguousarray(x[b].T).astype(bf16) for b in range(4)]
    in_maps = []
    for c in range(8):
        b, hh = divmod(c, 2)
        cs = slice(hh * 512, (hh + 1) * 512)
        wqkv = np.ascontiguousarray(
            np.concatenate(
                [qkv_w[:, 0:E][:, cs], qkv_w[:, E : 2 * E][:, cs], qkv_w[:, 2 * E :][:, cs]],
                axis=1,
            )
        ).astype(bf16)
        in_maps.append(
            {
                "xt": xts[b],
                "wqkv": wqkv,
                "bq": np.ascontiguousarray(qkv_b[0:E][cs]),
                "bv_bf": np.ascontiguousarray(qkv_b[2 * E :][cs]).astype(bf16),
                "wp": np.ascontiguousarray(proj_w[cs, :]).astype(bf16),
            }
        )
    return in_maps


def assemble_out(results, proj_b):
    proj_b = np.asarray(proj_b, dtype=np.float32)
    out = np.empty((4, SEQ, E), dtype=np.float32)
    for b in range(4):
        np.add(
            results[2 * b]["out"].astype(np.float32),
            results[2 * b + 1]["out"].astype(np.float32),
            out=out[b],
        )
        out[b] += proj_b
    return out


def run(inputs, trace=False, tmpdir=None):
    """Run on 8 NeuronCores; returns (output, BassKernelResults)."""
    from concourse.bass_utils import run_bass_kernel_spmd

    nc = build_nc()
    in_maps = make_in_maps(**inputs)
    res = run_bass_kernel_spmd(
        nc, in_maps, core_ids=list(range(8)), trace=trace, tmpdir=tmpdir
    )
    return assemble_out(res.results, inputs["proj_b"]), res


def kernel(x, qkv_w, qkv_b, proj_w, proj_b):
    out, _ = run(
        dict(x=x, qkv_w=qkv_w, qkv_b=qkv_b, proj_w=proj_w, proj_b=proj_b),
        trace=False,
    )
    return out


if __name__ == "__main__":
    rng = np.random.default_rng(0)
    x = rng.standard_normal((4, SEQ, E), dtype=np.float32)
    s = E ** -0.5
    inputs = dict(
        x=x,
        qkv_w=rng.standard_normal((E, 3 * E), dtype=np.float32) * s,
        qkv_b=rng.standard_normal((3 * E,), dtype=np.float32) * 0.02,
        proj_w=rng.standard_normal((E, E), dtype=np.float32) * s,
        proj_b=rng.standard_normal((E,), dtype=np.float32) * 0.02,
    )
    out = kernel(**inputs)
    print("out", out.shape, out.dtype, float(np.abs(out).mean()))
